# revision 12
# baseline (speedup 1.0000x reference)
"""Trainium2 Bass kernel for a single DecoderRNN step (LSTM cell + soft-dot
attention + vocab projection), SPMD over 8 NeuronCores.

Sharding: data-parallel over batch (16 rows/core) for the LSTM + attention,
tensor-parallel over vocab (4000 cols/core) for the decoder matmul, with one
on-device AllGather of h_tilde. Host only slices / transposes / casts.

Outputs (matching reference): (h_1 [128,1024], c_1 [128,1024],
alpha [128,512], logit [128,32000]).
"""

import contextlib
import os
import sys
import types

for _p in ("/opt/trn_rl_repo", "/root/.axon_site/_ro/trn_rl_repo"):
    if os.path.isdir(_p) and _p not in sys.path:
        sys.path.append(_p)

import numpy as np

import concourse.bass as bass
import concourse.mybir as mybir
import concourse.tile as tile
from concourse import bacc
from concourse.bass_utils import run_bass_kernel_spmd
from concourse.masks import make_identity


# NTFF profiling hook shim (used only when BASS_TRACE=1); missing in image.
def _install_ntff_shim():
    if "antenv.axon_hooks" in sys.modules:
        return
    try:
        import antenv  # noqa: F401
        from trn_agent_boot.trn_boot import _ntff_profile_via_ctypes
    except Exception:
        return
    so = "/opt/axon/libaxon_pjrt.so"
    hook = _ntff_profile_via_ctypes(so) if os.path.exists(so) else None
    mod = types.ModuleType("antenv.axon_hooks")
    mod.get_axon_ntff_profile_hook = lambda: hook
    mod.set_axon_ntff_profile_hook = lambda h: None
    sys.modules["antenv.axon_hooks"] = mod
    sys.modules["antenv"].axon_hooks = mod


_install_ntff_shim()

V, E, H, B, S = 32000, 512, 1024, 128, 512
NC = 8
BC = B // NC        # 16 batch rows per core
VC = V // NC        # 4000 vocab cols per core
NEG = -1e9
P = 128
G4 = 4 * H          # 4096 gate units
NV = 8              # vocab n-splits per core
NVW = VC // NV      # 500 cols per split
NKI = E // P        # 4 K-chunks from x
NKH = H // P        # 8 K-chunks from h0
NSPLIT = 8          # 512-col splits of 4096
NSC = S // P        # 4 s-chunks

F32 = mybir.dt.float32
# fp32r runs matmul rows 4x faster but at ~tf32 precision (~1e-3 end-to-end
# rel err measured on HW); fp32 measures ~1.4e-5. Default to exact.
MM_DT = mybir.dt.float32r if os.environ.get("KMM", "f32") == "f32r" else F32

DEBUG = bool(int(os.environ.get("KDEBUG", "0")))
# Dev bisect knob: 1=LSTM only, 2=+attention, 3=+wout/AG, 4=full (default)
STAGE = int(os.environ.get("KSTAGE", "4"))


class _StageStop(Exception):
    pass


def _f32(ap):
    """View an MM_DT AP as plain fp32 for vector/scalar-engine access."""
    if MM_DT is F32:
        return ap
    return ap.bitcast(F32)


def _build():
    nc = bacc.Bacc("TRN2", target_bir_lowering=False, debug=False, num_devices=NC)

    # ---- I/O ----
    idx = nc.dram_tensor("idx", [BC, 1], mybir.dt.int32, kind="ExternalInput")
    emb = nc.dram_tensor("emb", [V, E], F32, kind="ExternalInput")
    h0T = nc.dram_tensor("h0T", [H, BC], MM_DT, kind="ExternalInput")
    c0 = nc.dram_tensor("c0", [BC, H], F32, kind="ExternalInput")
    ctxd = nc.dram_tensor("ctxd", [BC, S, H], MM_DT, kind="ExternalInput")
    mnegT = nc.dram_tensor("mnegT", [S, BC], F32, kind="ExternalInput")
    # blocked weights: [n, k, 128, tile] so every (n, k) DMA is one
    # contiguous 256KB run
    wg_blk = nc.dram_tensor("wg_blk", [NSPLIT, NKI + NKH, P, 512], MM_DT,
                            kind="ExternalInput")
    bih = nc.dram_tensor("bih", [1, G4], F32, kind="ExternalInput")
    bhh = nc.dram_tensor("bhh", [1, G4], F32, kind="ExternalInput")
    win_blk = nc.dram_tensor("win_blk", [2, NKH, P, 512], MM_DT, kind="ExternalInput")
    wout_blk = nc.dram_tensor("wout_blk", [2, 2 * NKH, P, 512], MM_DT,
                              kind="ExternalInput")
    wdec_blk = nc.dram_tensor("wdec_blk", [NV, NKH, P, NVW], MM_DT,
                              kind="ExternalInput")
    bdec = nc.dram_tensor("bdec", [1, VC], F32, kind="ExternalInput")

    h1o = nc.dram_tensor("h1o", [BC, H], F32, kind="ExternalOutput")
    c1o = nc.dram_tensor("c1o", [BC, H], F32, kind="ExternalOutput")
    alphao = nc.dram_tensor("alphao", [BC, S], F32, kind="ExternalOutput")
    logito = nc.dram_tensor("logito", [B, VC], F32, kind="ExternalOutput")
    dbg = {}
    if DEBUG:
        dbg["x"] = nc.dram_tensor("dbg_x", [BC, E], F32, kind="ExternalOutput")
        dbg["gates"] = nc.dram_tensor("dbg_gates", [BC, G4], F32, kind="ExternalOutput")
        dbg["target"] = nc.dram_tensor("dbg_target", [BC, H], F32, kind="ExternalOutput")
        dbg["weighted"] = nc.dram_tensor("dbg_weighted", [BC, H], F32, kind="ExternalOutput")
        dbg["ht"] = nc.dram_tensor("dbg_ht", [BC, H], F32, kind="ExternalOutput")
        dbg["htfull"] = nc.dram_tensor("dbg_htfull", [B, H], F32, kind="ExternalOutput")

    with tile.TileContext(nc) as tc, contextlib.ExitStack() as ctx:
        pp = ctx.enter_context(tc.tile_pool(name="persist", bufs=1))
        wstream = ctx.enter_context(tc.tile_pool(name="wstream", bufs=4))
        ctxpool = ctx.enter_context(tc.tile_pool(name="ctxpool", bufs=3))
        small = ctx.enter_context(tc.tile_pool(name="small", bufs=2))
        psum = ctx.enter_context(tc.tile_pool(name="psum", bufs=1, space="PSUM"))
        dram = ctx.enter_context(tc.tile_pool(name="dram", bufs=1, space="DRAM"))

        def ptile(shape, name, dtype=F32):
            return pp.tile(shape, dtype, tag=name, name=name)

        try:
            # ---- constants ----
            ident = ptile([P, P], "ident")
            make_identity(nc, ident[:])
            ones_sq = ptile([P, P], "ones_sq")
            nc.gpsimd.memset(ones_sq[:], 1.0)

            # ---- index load + embedding gather ----
            idx_sb = ptile([BC, 1], "idx_sb", dtype=mybir.dt.int32)
            nc.sync.dma_start(idx_sb[:], idx[:])
            x_sb = ptile([BC, E], "x_sb")
            nc.gpsimd.indirect_dma_start(
                out=x_sb[:],
                out_offset=None,
                in_=emb[:],
                in_offset=bass.IndirectOffsetOnAxis(ap=idx_sb[:, :1], axis=0),
            )
            if DEBUG:
                nc.sync.dma_start(dbg["x"][:], x_sb[:])

            # ---- x^T (E on partitions) via PE transposes ----
            xT = ptile([P, NKI, BC], "xT", dtype=MM_DT)
            for k in range(NKI):
                pt = psum.tile([P, P], F32, tag="tr", bufs=2, name="pt_x")
                nc.tensor.transpose(pt[:, :BC], x_sb[:, k * P:(k + 1) * P], ident[:BC, :BC])
                nc.vector.tensor_copy(xT[:, k, :], pt[:, :BC])

            # ---- h0^T load ([H, BC] dram -> [128, 8, BC]) ----
            h0T_sb = ptile([P, NKH, BC], "h0T_sb", dtype=MM_DT)
            nc.sync.dma_start(h0T_sb[:], h0T[:].rearrange("(o p) b -> p o b", p=P))

            # ---- bias sum (b_ih + b_hh) in DRAM scratch via accumulate DMA ----
            bsumd = dram.tile([1, G4], F32, tag="bsumd", name="bsumd")
            nc.gpsimd.dma_start(bsumd[:], bih[:])
            nc.gpsimd.dma_start(bsumd[:], bhh[:], accum_op=mybir.AluOpType.add)

            # ---- LSTM gates: [BC, 4096] = x @ W_ih^T + h0 @ W_hh^T + b ----
            c0_sb = ptile([BC, H], "c0_sb")
            nc.sync.dma_start(c0_sb[:], c0[:])
            gact = [ptile([BC, H], f"gate{gi}") for gi in range(4)]
            func_by_gate = [
                mybir.ActivationFunctionType.Sigmoid,  # i
                mybir.ActivationFunctionType.Sigmoid,  # f
                mybir.ActivationFunctionType.Tanh,     # g
                mybir.ActivationFunctionType.Sigmoid,  # o
            ]
            for n in range(NSPLIT):
                pgn = psum.tile([BC, 512], F32, tag="acc", bufs=2, name="pg")
                cols = slice(n * 512, (n + 1) * 512)
                for k in range(NKI + NKH):
                    wt = wstream.tile([P, 512], MM_DT, tag="w", name="wt_g")
                    nc.sync.dma_start(wt[:], wg_blk[n, k])
                    lhsT = xT[:, k, :] if k < NKI else h0T_sb[:, k - NKI, :]
                    nc.tensor.matmul(
                        pgn[:], lhsT, wt[:],
                        start=(k == 0), stop=(k == NKI + NKH - 1),
                    )
                bsl = small.tile([1, 512], F32, tag="bs", name="bsl")
                nc.sync.dma_start(bsl[:], bsumd[:, cols])
                bb = small.tile([BC, 512], F32, tag="bb", name="bbt")
                nc.gpsimd.partition_broadcast(bb[:], bsl[:])
                gb = small.tile([BC, 512], F32, tag="gb", name="gbt")
                nc.vector.tensor_add(gb[:], pgn[:], bb[:])
                if DEBUG:
                    nc.sync.dma_start(dbg["gates"][:, cols], gb[:])
                gi, half = n // 2, n % 2
                nc.scalar.activation(
                    gact[gi][:, half * 512:(half + 1) * 512], gb[:], func_by_gate[gi]
                )

            # c1 = f*c0 + i*g ; h1 = o*tanh(c1)
            i_sb, f_sb, g_sb, o_sb = gact
            c1_sb = ptile([BC, H], "c1_sb")
            tmp_sb = ptile([BC, H], "tmp_sb")
            nc.vector.tensor_mul(c1_sb[:], f_sb[:], c0_sb[:])
            nc.vector.tensor_mul(tmp_sb[:], i_sb[:], g_sb[:])
            nc.vector.tensor_add(c1_sb[:], c1_sb[:], tmp_sb[:])
            th_sb = ptile([BC, H], "th_sb")
            nc.scalar.activation(th_sb[:], c1_sb[:], mybir.ActivationFunctionType.Tanh)
            h1_sb = ptile([BC, H], "h1_sb")
            nc.vector.tensor_mul(h1_sb[:], o_sb[:], th_sb[:])
            nc.sync.dma_start(h1o[:], h1_sb[:])
            nc.sync.dma_start(c1o[:], c1_sb[:])

            if STAGE < 2:
                raise _StageStop

            # ---- h1^T ----
            h1T = ptile([P, NKH, BC], "h1T", dtype=MM_DT)
            for k in range(NKH):
                pt = psum.tile([P, P], F32, tag="tr", bufs=2, name="pt_h1")
                nc.tensor.transpose(pt[:, :BC], h1_sb[:, k * P:(k + 1) * P], ident[:BC, :BC])
                nc.vector.tensor_copy(h1T[:, k, :], pt[:, :BC])

            # ---- target = h1 @ W_in^T  [BC, H] ----
            target_sb = ptile([BC, H], "target_sb")
            for n2 in range(2):
                cols = slice(n2 * 512, (n2 + 1) * 512)
                ptg = psum.tile([BC, 512], F32, tag="acc", bufs=2, name="ptg")
                for k in range(NKH):
                    wt = wstream.tile([P, 512], MM_DT, tag="w", name="wt_t")
                    nc.sync.dma_start(wt[:], win_blk[n2, k])
                    nc.tensor.matmul(
                        ptg[:], h1T[:, k, :], wt[:],
                        start=(k == 0), stop=(k == NKH - 1),
                    )
                nc.scalar.copy(target_sb[:, cols], ptg[:])
            if DEBUG:
                nc.sync.dma_start(dbg["target"][:], target_sb[:])

            # ---- masked scores + softmax + weighted sum, per batch row ----
            mneg_sb = ptile([P, NSC, BC], "mneg_sb")
            nc.sync.dma_start(mneg_sb[:], mnegT[:].rearrange("(o p) b -> p o b", p=P))

            alpha_cols = ptile([P, NSC, BC], "alpha_cols")   # alpha, s on partitions
            nc.gpsimd.memset(alpha_cols[:], 0.0)
            weighted_sb = ptile([BC, H], "weighted_sb")
            junk = small.tile([P, H], F32, tag="junk", bufs=1, name="junk")
            # one shared PSUM accumulator: each b contributes only its own row
            # (its alpha tile is zero in every other column), so after the loop
            # all BC rows are valid and a single aligned copy suffices.
            pw = psum.tile([BC, H], F32, tag="w", bufs=1, name="pwt")

            for b in range(BC):
                # broadcast target row b across partitions (via partition 0:
                # partition_broadcast requires input anchored at partition 0)
                tb0 = small.tile([1, H], F32, tag="tb0", bufs=3, name="tb0t")
                nc.sync.dma_start(tb0[:], target_sb[b:b + 1, :])
                tb = small.tile([P, H], F32, tag="tb", bufs=3, name="tbt")
                nc.gpsimd.partition_broadcast(tb[:], tb0[:])
                # ctx rows for batch b: one DMA, [128, 4, 1024] (4KB runs)
                ct = ctxpool.tile([P, NSC, H], MM_DT, tag="ctx", name="ct")
                nc.sync.dma_start(
                    ct[:], ctxd[b].rearrange("(c p) h -> p c h", p=P)
                )
                # scores[s] = sum_h ctx[s,h] * target[h]  (+ mask bias)
                sc = small.tile([P, NSC], F32, tag="sc", bufs=4, name="sct")
                for c in range(NSC):
                    # NB: InstTensorTensorReduce faults at runtime on this
                    # stack; InstTensorScalarPtr(is_scalar_tensor_tensor) with
                    # accum_out is the working fused multiply-reduce.
                    nc.vector.scalar_tensor_tensor(
                        out=junk[:],
                        in0=_f32(ct[:, c, :]),
                        scalar=0.0,
                        in1=tb[:],
                        op0=mybir.AluOpType.add,
                        op1=mybir.AluOpType.mult,
                        accum_out=sc[:, c:c + 1],
                    )
                nc.vector.tensor_add(sc[:], sc[:], mneg_sb[:, :, b])
                # softmax over all 512 entries (s on partitions x 4 chunks).
                # No max-subtraction: |scores| << 80 for this model family, so
                # exp/sum in fp32 matches the reference to ~1e-7 relative.
                pe_sb = small.tile([P, NSC], F32, tag="pe", bufs=4, name="pet")
                nc.scalar.activation(pe_sb[:], sc[:], mybir.ActivationFunctionType.Exp)
                # partition sums broadcast back to every partition: ones^T @ pe
                psm = psum.tile([P, NSC], F32, tag="sm", bufs=1, name="psm")
                nc.tensor.matmul(psm[:], ones_sq[:], pe_sb[:], start=True, stop=True)
                tot = small.tile([P, 1], F32, tag="tot", bufs=4, name="tott")
                nc.vector.tensor_reduce(
                    tot[:], psm[:], axis=mybir.AxisListType.X, op=mybir.AluOpType.add
                )
                rinv = small.tile([P, 1], F32, tag="rinv", bufs=4, name="rinvt")
                nc.vector.reciprocal(rinv[:], tot[:])
                nc.vector.tensor_scalar_mul(alpha_cols[:, :, b], pe_sb[:], rinv[:])
                # fresh zeroed alpha tile with only column b set -> block-diag
                ab = small.tile([P, NSC, BC], MM_DT, tag="ab", bufs=3, name="abt")
                nc.gpsimd.memset(_f32(ab[:]), 0.0)  # memset can't target f32r
                nc.vector.tensor_scalar_mul(ab[:, :, b], pe_sb[:], rinv[:])
                # weighted[b] = sum_s alpha[s] * ctx[s, :]
                for c in range(NSC):
                    for nh in range(2):
                        nc.tensor.matmul(
                            pw[:, nh * 512:(nh + 1) * 512],
                            ab[:, c, :],
                            ct[:, c, nh * 512:(nh + 1) * 512],
                            start=(b == 0 and c == 0),
                            stop=(b == BC - 1 and c == NSC - 1),
                            skip_group_check=True,
                        )
            nc.scalar.copy(weighted_sb[:], pw[:])
            if DEBUG:
                nc.sync.dma_start(dbg["weighted"][:], weighted_sb[:])

            # ---- alpha output (transpose alpha_cols -> [BC, S]) ----
            alphaT = ptile([BC, S], "alphaT")
            for c in range(NSC):
                pt = psum.tile([P, P], F32, tag="tr", bufs=2, name="pt_a")
                nc.tensor.transpose(pt[:BC, :], alpha_cols[:, c, :], ident[:])
                nc.vector.tensor_copy(alphaT[:, c * P:(c + 1) * P], pt[:BC, :])
            nc.sync.dma_start(alphao[:], alphaT[:])

            if STAGE < 3:
                raise _StageStop

            # ---- weighted^T ----
            wgtT = ptile([P, NKH, BC], "wgtT", dtype=MM_DT)
            for k in range(NKH):
                pt = psum.tile([P, P], F32, tag="tr", bufs=2, name="pt_w")
                nc.tensor.transpose(pt[:, :BC], weighted_sb[:, k * P:(k + 1) * P],
                                    ident[:BC, :BC])
                nc.vector.tensor_copy(wgtT[:, k, :], pt[:, :BC])

            # ---- h_tilde = tanh([weighted, h1] @ W_out^T)  [BC, H] ----
            ht_sb = ptile([BC, H], "ht_sb")
            for n2 in range(2):
                cols = slice(n2 * 512, (n2 + 1) * 512)
                ptg = psum.tile([BC, 512], F32, tag="acc", bufs=2, name="pto")
                for k in range(2 * NKH):   # 16 K-chunks: 0-7 weighted, 8-15 h1
                    wt = wstream.tile([P, 512], MM_DT, tag="w", name="wt_o")
                    nc.sync.dma_start(wt[:], wout_blk[n2, k])
                    lhsT = wgtT[:, k, :] if k < NKH else h1T[:, k - NKH, :]
                    nc.tensor.matmul(
                        ptg[:], lhsT, wt[:],
                        start=(k == 0), stop=(k == 2 * NKH - 1),
                    )
                nc.scalar.activation(
                    ht_sb[:, cols], ptg[:], mybir.ActivationFunctionType.Tanh
                )
            if DEBUG:
                nc.sync.dma_start(dbg["ht"][:], ht_sb[:])

            # ---- AllGather h_tilde across the 8 cores ----
            htd = dram.tile([BC, H], F32, tag="htd", name="htd")
            htg = dram.tile([B, H], F32, addr_space="Shared", tag="htg", name="htg")
            nc.sync.dma_start(htd[:], ht_sb[:])
            nc.gpsimd.collective_compute(
                "AllGather",
                mybir.AluOpType.bypass,
                replica_groups=[list(range(NC))],
                ins=[htd.opt()],
                outs=[htg.opt()],
            )
            htfull = ptile([B, H], "htfull")
            nc.sync.dma_start(htfull[:], htg[:])
            if DEBUG:
                nc.sync.dma_start(dbg["htfull"][:], htfull[:])

            if STAGE < 4:
                raise _StageStop

            # ---- h_tilde^T (full batch) ----
            htT = ptile([P, NKH, B], "htT", dtype=MM_DT)
            for k in range(NKH):
                pt = psum.tile([P, P], F32, tag="tr", bufs=2, name="pt_ht")
                nc.tensor.transpose(pt[:], htfull[:, k * P:(k + 1) * P], ident[:])
                nc.vector.tensor_copy(htT[:, k, :], pt[:])

            # ---- logits shard: [B, VC] = htfull @ W_dec^T + b_dec ----
            # wd slices prefetch during attention/AllGather (16 bufs = 4MB)
            for nv in range(NV):
                cols = slice(nv * NVW, (nv + 1) * NVW)
                pl = psum.tile([B, NVW], F32, tag="pl", bufs=1, name="pl")
                for k in range(NKH):
                    wt = wstream.tile([P, NVW], MM_DT, tag="wd", bufs=16, name="wt_d")
                    nc.sync.dma_start(wt[:], wdec_blk[nv, k])
                    nc.tensor.matmul(
                        pl[:], htT[:, k, :], wt[:],
                        start=(k == 0), stop=(k == NKH - 1),
                    )
                bsl = small.tile([1, NVW], F32, tag="bds", name="bsld")
                nc.sync.dma_start(bsl[:], bdec[:, cols])
                bb = small.tile([B, NVW], F32, tag="bdb", name="bbd")
                nc.gpsimd.partition_broadcast(bb[:], bsl[:])
                lg = small.tile([B, NVW], F32, tag="lg", name="lgt")
                nc.vector.tensor_add(lg[:], pl[:], bb[:])
                nc.sync.dma_start(logito[:, cols], lg[:])
        except _StageStop:
            pass

    nc.compile()
    return nc


_CACHE = {}
last_results = None


def _get_program():
    key = (MM_DT, DEBUG, STAGE)
    if key not in _CACHE:
        _CACHE[key] = _build()
    return _CACHE[key]


def kernel(previous_word, h_0, c_0, ctx, ctx_mask, emb, W_ih, W_hh, b_ih, b_hh,
           W_in, W_out, W_dec, b_dec):
    global last_results
    f32 = np.float32
    previous_word = np.asarray(previous_word)
    idx_all = np.ascontiguousarray(previous_word.reshape(B, 1).astype(np.int32))
    h_0 = np.asarray(h_0, dtype=f32)
    c_0 = np.asarray(c_0, dtype=f32)
    ctx = np.asarray(ctx, dtype=f32)
    mask_neg = np.where(np.asarray(ctx_mask), f32(NEG), f32(0.0)).astype(f32)
    emb = np.ascontiguousarray(np.asarray(emb, dtype=f32))
    W_ih = np.asarray(W_ih, dtype=f32)
    W_hh = np.asarray(W_hh, dtype=f32)
    # blocked layouts: [n, k, 128, 512], each block one contiguous DMA
    wih_blk = W_ih.T.reshape(NKI, P, NSPLIT, 512).transpose(2, 0, 1, 3)
    whh_blk = W_hh.T.reshape(NKH, P, NSPLIT, 512).transpose(2, 0, 1, 3)
    wg_blk = np.ascontiguousarray(np.concatenate([wih_blk, whh_blk], axis=1))
    bih = np.asarray(b_ih, dtype=f32).reshape(1, G4)
    bhh = np.asarray(b_hh, dtype=f32).reshape(1, G4)
    win_blk = np.ascontiguousarray(
        np.asarray(W_in, dtype=f32).T.reshape(NKH, P, 2, 512).transpose(2, 0, 1, 3))
    wout_blk = np.ascontiguousarray(
        np.asarray(W_out, dtype=f32).T.reshape(2 * NKH, P, 2, 512).transpose(2, 0, 1, 3))
    W_dec = np.asarray(W_dec, dtype=f32)
    b_dec = np.asarray(b_dec, dtype=f32)

    nc = _get_program()

    in_maps = []
    for c in range(NC):
        rows = slice(c * BC, (c + 1) * BC)
        vs = slice(c * VC, (c + 1) * VC)
        wdec_blk = np.ascontiguousarray(
            W_dec[vs].T.reshape(NKH, P, NV, NVW).transpose(2, 0, 1, 3))
        in_maps.append({
            "idx": idx_all[rows],
            "emb": emb,
            "h0T": np.ascontiguousarray(h_0[rows].T),
            "c0": np.ascontiguousarray(c_0[rows]),
            "ctxd": np.ascontiguousarray(ctx[rows]),
            "mnegT": np.ascontiguousarray(mask_neg[rows].T),
            "wg_blk": wg_blk,
            "bih": bih,
            "bhh": bhh,
            "win_blk": win_blk,
            "wout_blk": wout_blk,
            "wdec_blk": wdec_blk,
            "bdec": np.ascontiguousarray(b_dec[vs].reshape(1, VC)),
        })

    res = run_bass_kernel_spmd(nc, in_maps, list(range(NC)))
    last_results = res
    r = res.results
    h_1 = np.concatenate([r[c]["h1o"] for c in range(NC)], axis=0)
    c_1 = np.concatenate([r[c]["c1o"] for c in range(NC)], axis=0)
    alpha = np.concatenate([r[c]["alphao"] for c in range(NC)], axis=0)
    logit = np.concatenate([r[c]["logito"] for c in range(NC)], axis=1)
    return h_1, c_1, alpha, logit


# revision 14
# speedup vs baseline: 1.0210x; 1.0210x over previous
"""Trainium2 Bass kernel for a single DecoderRNN step (LSTM cell + soft-dot
attention + vocab projection), SPMD over 8 NeuronCores.

Sharding: data-parallel over batch (16 rows/core) for the LSTM + attention,
tensor-parallel over vocab (4000 cols/core) for the decoder matmul, with one
on-device AllGather of h_tilde. Host only slices / transposes / casts.

Outputs (matching reference): (h_1 [128,1024], c_1 [128,1024],
alpha [128,512], logit [128,32000]).
"""

import contextlib
import os
import sys
import types

for _p in ("/opt/trn_rl_repo", "/root/.axon_site/_ro/trn_rl_repo"):
    if os.path.isdir(_p) and _p not in sys.path:
        sys.path.append(_p)

import numpy as np

import concourse.bass as bass
import concourse.mybir as mybir
import concourse.tile as tile
from concourse import bacc
from concourse.bass_utils import run_bass_kernel_spmd
from concourse.masks import make_identity


# NTFF profiling hook shim (used only when BASS_TRACE=1); missing in image.
def _install_ntff_shim():
    if "antenv.axon_hooks" in sys.modules:
        return
    try:
        import antenv  # noqa: F401
        from trn_agent_boot.trn_boot import _ntff_profile_via_ctypes
    except Exception:
        return
    so = "/opt/axon/libaxon_pjrt.so"
    hook = _ntff_profile_via_ctypes(so) if os.path.exists(so) else None
    mod = types.ModuleType("antenv.axon_hooks")
    mod.get_axon_ntff_profile_hook = lambda: hook
    mod.set_axon_ntff_profile_hook = lambda h: None
    sys.modules["antenv.axon_hooks"] = mod
    sys.modules["antenv"].axon_hooks = mod


_install_ntff_shim()

V, E, H, B, S = 32000, 512, 1024, 128, 512
NC = 8
BC = B // NC        # 16 batch rows per core
VC = V // NC        # 4000 vocab cols per core
NEG = -1e9
P = 128
G4 = 4 * H          # 4096 gate units
NV = 8              # vocab n-splits per core
NVW = VC // NV      # 500 cols per split
NKI = E // P        # 4 K-chunks from x
NKH = H // P        # 8 K-chunks from h0
NSPLIT = 8          # 512-col splits of 4096
NSC = S // P        # 4 s-chunks

F32 = mybir.dt.float32
# fp32r runs matmul rows 4x faster but at ~tf32 precision (~1e-3 end-to-end
# rel err measured on HW); fp32 measures ~1.4e-5. Default to exact.
MM_DT = mybir.dt.float32r if os.environ.get("KMM", "f32") == "f32r" else F32

DEBUG = bool(int(os.environ.get("KDEBUG", "0")))
# Dev bisect knob: 1=LSTM only, 2=+attention, 3=+wout/AG, 4=full (default)
STAGE = int(os.environ.get("KSTAGE", "4"))


class _StageStop(Exception):
    pass


def _f32(ap):
    """View an MM_DT AP as plain fp32 for vector/scalar-engine access."""
    if MM_DT is F32:
        return ap
    return ap.bitcast(F32)


def _build():
    nc = bacc.Bacc("TRN2", target_bir_lowering=False, debug=False, num_devices=NC)

    # ---- I/O ----
    idx = nc.dram_tensor("idx", [BC, 1], mybir.dt.int32, kind="ExternalInput")
    emb = nc.dram_tensor("emb", [V, E], F32, kind="ExternalInput")
    h0T = nc.dram_tensor("h0T", [H, BC], MM_DT, kind="ExternalInput")
    c0 = nc.dram_tensor("c0", [BC, H], F32, kind="ExternalInput")
    ctxd = nc.dram_tensor("ctxd", [BC, S, H], MM_DT, kind="ExternalInput")
    mnegT = nc.dram_tensor("mnegT", [S, BC], F32, kind="ExternalInput")
    # blocked weights: [n, k, 128, tile] so every (n, k) DMA is one
    # contiguous 256KB run
    wg_blk = nc.dram_tensor("wg_blk", [NSPLIT, NKI + NKH, P, 512], MM_DT,
                            kind="ExternalInput")
    bih = nc.dram_tensor("bih", [1, G4], F32, kind="ExternalInput")
    bhh = nc.dram_tensor("bhh", [1, G4], F32, kind="ExternalInput")
    win_blk = nc.dram_tensor("win_blk", [2, NKH, P, 512], MM_DT, kind="ExternalInput")
    wout_blk = nc.dram_tensor("wout_blk", [2, 2 * NKH, P, 512], MM_DT,
                              kind="ExternalInput")
    wdec_blk = nc.dram_tensor("wdec_blk", [NV, NKH, P, NVW], MM_DT,
                              kind="ExternalInput")
    bdec = nc.dram_tensor("bdec", [1, VC], F32, kind="ExternalInput")

    h1o = nc.dram_tensor("h1o", [BC, H], F32, kind="ExternalOutput")
    c1o = nc.dram_tensor("c1o", [BC, H], F32, kind="ExternalOutput")
    alphao = nc.dram_tensor("alphao", [BC, S], F32, kind="ExternalOutput")
    logito = nc.dram_tensor("logito", [B, VC], F32, kind="ExternalOutput")
    dbg = {}
    if DEBUG:
        dbg["x"] = nc.dram_tensor("dbg_x", [BC, E], F32, kind="ExternalOutput")
        dbg["gates"] = nc.dram_tensor("dbg_gates", [BC, G4], F32, kind="ExternalOutput")
        dbg["target"] = nc.dram_tensor("dbg_target", [BC, H], F32, kind="ExternalOutput")
        dbg["weighted"] = nc.dram_tensor("dbg_weighted", [BC, H], F32, kind="ExternalOutput")
        dbg["ht"] = nc.dram_tensor("dbg_ht", [BC, H], F32, kind="ExternalOutput")
        dbg["htfull"] = nc.dram_tensor("dbg_htfull", [B, H], F32, kind="ExternalOutput")

    with tile.TileContext(nc) as tc, contextlib.ExitStack() as ctx:
        pp = ctx.enter_context(tc.tile_pool(name="persist", bufs=1))
        wstream = ctx.enter_context(tc.tile_pool(name="wstream", bufs=4))
        ctxpool = ctx.enter_context(tc.tile_pool(name="ctxpool", bufs=2))
        small = ctx.enter_context(tc.tile_pool(name="small", bufs=2))
        psum = ctx.enter_context(tc.tile_pool(name="psum", bufs=1, space="PSUM"))
        dram = ctx.enter_context(tc.tile_pool(name="dram", bufs=1, space="DRAM"))

        def ptile(shape, name, dtype=F32):
            return pp.tile(shape, dtype, tag=name, name=name)

        try:
            # ---- constants ----
            ident = ptile([P, P], "ident")
            make_identity(nc, ident[:])
            ones_sq = ptile([P, P], "ones_sq")
            nc.gpsimd.memset(ones_sq[:], 1.0)

            # ---- index load + embedding gather ----
            idx_sb = ptile([BC, 1], "idx_sb", dtype=mybir.dt.int32)
            nc.sync.dma_start(idx_sb[:], idx[:])
            x_sb = ptile([BC, E], "x_sb")
            nc.gpsimd.indirect_dma_start(
                out=x_sb[:],
                out_offset=None,
                in_=emb[:],
                in_offset=bass.IndirectOffsetOnAxis(ap=idx_sb[:, :1], axis=0),
            )
            if DEBUG:
                nc.sync.dma_start(dbg["x"][:], x_sb[:])

            # ---- x^T (E on partitions) via PE transposes ----
            xT = ptile([P, NKI, BC], "xT", dtype=MM_DT)
            for k in range(NKI):
                pt = psum.tile([P, P], F32, tag="tr", bufs=2, name="pt_x")
                nc.tensor.transpose(pt[:, :BC], x_sb[:, k * P:(k + 1) * P], ident[:BC, :BC])
                nc.vector.tensor_copy(xT[:, k, :], pt[:, :BC])

            # ---- h0^T load ([H, BC] dram -> [128, 8, BC]) ----
            h0T_sb = ptile([P, NKH, BC], "h0T_sb", dtype=MM_DT)
            nc.sync.dma_start(h0T_sb[:], h0T[:].rearrange("(o p) b -> p o b", p=P))

            # ---- bias sum (b_ih + b_hh) in DRAM scratch via accumulate DMA ----
            bsumd = dram.tile([1, G4], F32, tag="bsumd", name="bsumd")
            nc.gpsimd.dma_start(bsumd[:], bih[:])
            nc.gpsimd.dma_start(bsumd[:], bhh[:], accum_op=mybir.AluOpType.add)

            # ---- LSTM gates: [BC, 4096] = x @ W_ih^T + h0 @ W_hh^T + b ----
            c0_sb = ptile([BC, H], "c0_sb")
            nc.sync.dma_start(c0_sb[:], c0[:])
            gact = [ptile([BC, H], f"gate{gi}") for gi in range(4)]
            func_by_gate = [
                mybir.ActivationFunctionType.Sigmoid,  # i
                mybir.ActivationFunctionType.Sigmoid,  # f
                mybir.ActivationFunctionType.Tanh,     # g
                mybir.ActivationFunctionType.Sigmoid,  # o
            ]
            NKG = NKI + NKH      # 12 contraction chunks per gate split
            for n in range(NSPLIT):
                pgn = psum.tile([BC, 512], F32, tag="acc", bufs=2, name="pg")
                cols = slice(n * 512, (n + 1) * 512)
                for kg in range(NKG // 4):   # 3 grouped loads of 4 chunks
                    wt = wstream.tile([P, 4, 512], MM_DT, tag="w", name="wt_g")
                    nc.sync.dma_start(wt[:], wg_blk[n, kg * 4:(kg + 1) * 4].rearrange("k p j -> p k j"))
                    for kk in range(4):
                        k = kg * 4 + kk
                        lhsT = xT[:, k, :] if k < NKI else h0T_sb[:, k - NKI, :]
                        nc.tensor.matmul(
                            pgn[:], lhsT, wt[:, kk, :],
                            start=(k == 0), stop=(k == NKG - 1),
                        )
                bsl = small.tile([1, 512], F32, tag="bs", name="bsl")
                nc.sync.dma_start(bsl[:], bsumd[:, cols])
                bb = small.tile([BC, 512], F32, tag="bb", name="bbt")
                nc.gpsimd.partition_broadcast(bb[:], bsl[:])
                gb = small.tile([BC, 512], F32, tag="gb", name="gbt")
                nc.vector.tensor_add(gb[:], pgn[:], bb[:])
                if DEBUG:
                    nc.sync.dma_start(dbg["gates"][:, cols], gb[:])
                gi, half = n // 2, n % 2
                nc.scalar.activation(
                    gact[gi][:, half * 512:(half + 1) * 512], gb[:], func_by_gate[gi]
                )

            # c1 = f*c0 + i*g ; h1 = o*tanh(c1)
            i_sb, f_sb, g_sb, o_sb = gact
            c1_sb = ptile([BC, H], "c1_sb")
            th_sb = ptile([BC, H], "th_sb")
            nc.vector.tensor_mul(c1_sb[:], f_sb[:], c0_sb[:])
            nc.vector.tensor_mul(th_sb[:], i_sb[:], g_sb[:])
            nc.vector.tensor_add(c1_sb[:], c1_sb[:], th_sb[:])
            nc.scalar.activation(th_sb[:], c1_sb[:], mybir.ActivationFunctionType.Tanh)
            h1_sb = ptile([BC, H], "h1_sb")
            nc.vector.tensor_mul(h1_sb[:], o_sb[:], th_sb[:])
            nc.sync.dma_start(h1o[:], h1_sb[:])
            nc.sync.dma_start(c1o[:], c1_sb[:])

            if STAGE < 2:
                raise _StageStop

            # ---- h1^T ----
            h1T = ptile([P, NKH, BC], "h1T", dtype=MM_DT)
            for k in range(NKH):
                pt = psum.tile([P, P], F32, tag="tr", bufs=2, name="pt_h1")
                nc.tensor.transpose(pt[:, :BC], h1_sb[:, k * P:(k + 1) * P], ident[:BC, :BC])
                nc.vector.tensor_copy(h1T[:, k, :], pt[:, :BC])

            # ---- target = h1 @ W_in^T  [BC, H] ----
            target_sb = ptile([BC, H], "target_sb")
            for n2 in range(2):
                cols = slice(n2 * 512, (n2 + 1) * 512)
                ptg = psum.tile([BC, 512], F32, tag="acc", bufs=2, name="ptg")
                for kg in range(NKH // 4):
                    wt = wstream.tile([P, 4, 512], MM_DT, tag="w", name="wt_t")
                    nc.sync.dma_start(wt[:], win_blk[n2, kg * 4:(kg + 1) * 4].rearrange("k p j -> p k j"))
                    for kk in range(4):
                        k = kg * 4 + kk
                        nc.tensor.matmul(
                            ptg[:], h1T[:, k, :], wt[:, kk, :],
                            start=(k == 0), stop=(k == NKH - 1),
                        )
                nc.scalar.copy(target_sb[:, cols], ptg[:])
            if DEBUG:
                nc.sync.dma_start(dbg["target"][:], target_sb[:])

            # ---- masked scores + softmax + weighted sum, per batch row ----
            mneg_sb = ptile([P, NSC, BC], "mneg_sb")
            nc.sync.dma_start(mneg_sb[:], mnegT[:].rearrange("(o p) b -> p o b", p=P))

            alpha_cols = ptile([P, NSC, BC], "alpha_cols")   # alpha, s on partitions
            nc.gpsimd.memset(alpha_cols[:], 0.0)
            zeros_ab = ptile([P, NSC, BC], "zeros_ab")
            nc.gpsimd.memset(zeros_ab[:], 0.0)
            weighted_sb = ptile([BC, H], "weighted_sb")
            junk = small.tile([P, H], F32, tag="junk", bufs=1, name="junk")
            # one shared PSUM accumulator: each b contributes only its own row
            # (its alpha tile is zero in every other column), so after the loop
            # all BC rows are valid and a single aligned copy suffices.
            pw = psum.tile([BC, H], F32, tag="w", bufs=1, name="pwt")

            for b in range(BC):
                # broadcast target row b across partitions (via partition 0:
                # partition_broadcast requires input anchored at partition 0)
                tb0 = small.tile([1, H], F32, tag="tb0", bufs=3, name="tb0t")
                nc.sync.dma_start(tb0[:], target_sb[b:b + 1, :])
                tb = small.tile([P, H], F32, tag="tb", bufs=3, name="tbt")
                nc.gpsimd.partition_broadcast(tb[:], tb0[:])
                # ctx rows for batch b: one DMA, [128, 4, 1024] (4KB runs)
                ct = ctxpool.tile([P, NSC, H], MM_DT, tag="ctx", name="ct")
                nc.sync.dma_start(
                    ct[:], ctxd[b].rearrange("(c p) h -> p c h", p=P)
                )
                # scores[s] = sum_h ctx[s,h] * target[h]  (+ mask bias)
                sc = small.tile([P, NSC], F32, tag="sc", bufs=4, name="sct")
                for c in range(NSC):
                    # NB: InstTensorTensorReduce faults at runtime on this
                    # stack; InstTensorScalarPtr(is_scalar_tensor_tensor) with
                    # accum_out is the working fused multiply-reduce.
                    nc.vector.scalar_tensor_tensor(
                        out=junk[:],
                        in0=_f32(ct[:, c, :]),
                        scalar=0.0,
                        in1=tb[:],
                        op0=mybir.AluOpType.add,
                        op1=mybir.AluOpType.mult,
                        accum_out=sc[:, c:c + 1],
                    )
                nc.vector.tensor_add(sc[:], sc[:], mneg_sb[:, :, b])
                # softmax over all 512 entries (s on partitions x 4 chunks).
                # No max-subtraction: |scores| << 80 for this model family, so
                # exp/sum in fp32 matches the reference to ~1e-7 relative.
                pe_sb = small.tile([P, NSC], F32, tag="pe", bufs=4, name="pet")
                nc.scalar.activation(pe_sb[:], sc[:], mybir.ActivationFunctionType.Exp)
                # partition sums broadcast back to every partition: ones^T @ pe
                psm = psum.tile([P, NSC], F32, tag="sm", bufs=1, name="psm")
                nc.tensor.matmul(psm[:], ones_sq[:], pe_sb[:], start=True, stop=True)
                tot = small.tile([P, 1], F32, tag="tot", bufs=4, name="tott")
                nc.vector.tensor_reduce(
                    tot[:], psm[:], axis=mybir.AxisListType.X, op=mybir.AluOpType.add
                )
                rinv = small.tile([P, 1], F32, tag="rinv", bufs=4, name="rinvt")
                nc.vector.reciprocal(rinv[:], tot[:])
                nc.vector.tensor_scalar_mul(alpha_cols[:, :, b], pe_sb[:], rinv[:])
                # fresh zeroed alpha tile with only column b set -> block-diag
                ab = small.tile([P, NSC, BC], MM_DT, tag="ab", bufs=3, name="abt")
                nc.vector.tensor_copy(_f32(ab[:]), zeros_ab[:])  # keep GPSIMD free
                nc.vector.tensor_scalar_mul(ab[:, :, b], pe_sb[:], rinv[:])
                # weighted[b] = sum_s alpha[s] * ctx[s, :]
                for c in range(NSC):
                    for nh in range(2):
                        nc.tensor.matmul(
                            pw[:, nh * 512:(nh + 1) * 512],
                            ab[:, c, :],
                            ct[:, c, nh * 512:(nh + 1) * 512],
                            start=(b == 0 and c == 0),
                            stop=(b == BC - 1 and c == NSC - 1),
                            skip_group_check=True,
                        )
            nc.scalar.copy(weighted_sb[:], pw[:])
            if DEBUG:
                nc.sync.dma_start(dbg["weighted"][:], weighted_sb[:])

            # ---- alpha output (transpose alpha_cols -> [BC, S]) ----
            alphaT = ptile([BC, S], "alphaT")
            for c in range(NSC):
                pt = psum.tile([P, P], F32, tag="tr", bufs=2, name="pt_a")
                nc.tensor.transpose(pt[:BC, :], alpha_cols[:, c, :], ident[:])
                nc.vector.tensor_copy(alphaT[:, c * P:(c + 1) * P], pt[:BC, :])
            nc.sync.dma_start(alphao[:], alphaT[:])

            if STAGE < 3:
                raise _StageStop

            # ---- weighted^T ----
            wgtT = ptile([P, NKH, BC], "wgtT", dtype=MM_DT)
            for k in range(NKH):
                pt = psum.tile([P, P], F32, tag="tr", bufs=2, name="pt_w")
                nc.tensor.transpose(pt[:, :BC], weighted_sb[:, k * P:(k + 1) * P],
                                    ident[:BC, :BC])
                nc.vector.tensor_copy(wgtT[:, k, :], pt[:, :BC])

            # ---- h_tilde = tanh([weighted, h1] @ W_out^T)  [BC, H] ----
            ht_sb = ptile([BC, H], "ht_sb")
            for n2 in range(2):
                cols = slice(n2 * 512, (n2 + 1) * 512)
                ptg = psum.tile([BC, 512], F32, tag="acc", bufs=2, name="pto")
                for kg in range(2 * NKH // 4):  # chunks 0-7 weighted, 8-15 h1
                    wt = wstream.tile([P, 4, 512], MM_DT, tag="w", name="wt_o")
                    nc.sync.dma_start(wt[:], wout_blk[n2, kg * 4:(kg + 1) * 4].rearrange("k p j -> p k j"))
                    for kk in range(4):
                        k = kg * 4 + kk
                        lhsT = wgtT[:, k, :] if k < NKH else h1T[:, k - NKH, :]
                        nc.tensor.matmul(
                            ptg[:], lhsT, wt[:, kk, :],
                            start=(k == 0), stop=(k == 2 * NKH - 1),
                        )
                nc.scalar.activation(
                    ht_sb[:, cols], ptg[:], mybir.ActivationFunctionType.Tanh
                )
            if DEBUG:
                nc.sync.dma_start(dbg["ht"][:], ht_sb[:])

            # ---- AllGather h_tilde across the 8 cores ----
            htd = dram.tile([BC, H], F32, tag="htd", name="htd")
            htg = dram.tile([B, H], F32, addr_space="Shared", tag="htg", name="htg")
            nc.sync.dma_start(htd[:], ht_sb[:])
            nc.gpsimd.collective_compute(
                "AllGather",
                mybir.AluOpType.bypass,
                replica_groups=[list(range(NC))],
                ins=[htd.opt()],
                outs=[htg.opt()],
            )
            htfull = ptile([B, H], "htfull")
            nc.sync.dma_start(htfull[:], htg[:])
            if DEBUG:
                nc.sync.dma_start(dbg["htfull"][:], htfull[:])

            if STAGE < 4:
                raise _StageStop

            # ---- h_tilde^T (full batch) ----
            htT = ptile([P, NKH, B], "htT", dtype=MM_DT)
            for k in range(NKH):
                pt = psum.tile([P, P], F32, tag="tr", bufs=2, name="pt_ht")
                nc.tensor.transpose(pt[:], htfull[:, k * P:(k + 1) * P], ident[:])
                nc.vector.tensor_copy(htT[:, k, :], pt[:])

            # ---- logits shard: [B, VC] = htfull @ W_dec^T + b_dec ----
            # wd slices prefetch during attention/AllGather (16 bufs = 4MB)
            for nv in range(NV):
                cols = slice(nv * NVW, (nv + 1) * NVW)
                pl = psum.tile([B, NVW], F32, tag="pl", bufs=1, name="pl")
                for kg in range(NKH // 4):
                    wt = wstream.tile([P, 4, NVW], MM_DT, tag="wd", bufs=3, name="wt_d")
                    nc.sync.dma_start(wt[:], wdec_blk[nv, kg * 4:(kg + 1) * 4].rearrange("k p j -> p k j"))
                    for kk in range(4):
                        k = kg * 4 + kk
                        nc.tensor.matmul(
                            pl[:], htT[:, k, :], wt[:, kk, :],
                            start=(k == 0), stop=(k == NKH - 1),
                        )
                bsl = small.tile([1, NVW], F32, tag="bs", name="bsld")
                nc.sync.dma_start(bsl[:], bdec[:, cols])
                bb = small.tile([B, NVW], F32, tag="bb", name="bbd")
                nc.gpsimd.partition_broadcast(bb[:], bsl[:])
                lg = small.tile([B, NVW], F32, tag="gb", name="lgt")
                nc.vector.tensor_add(lg[:], pl[:], bb[:])
                nc.sync.dma_start(logito[:, cols], lg[:])
        except _StageStop:
            pass

    nc.compile()
    return nc


_CACHE = {}
last_results = None


def _get_program():
    key = (MM_DT, DEBUG, STAGE)
    if key not in _CACHE:
        _CACHE[key] = _build()
    return _CACHE[key]


def kernel(previous_word, h_0, c_0, ctx, ctx_mask, emb, W_ih, W_hh, b_ih, b_hh,
           W_in, W_out, W_dec, b_dec):
    global last_results
    f32 = np.float32
    previous_word = np.asarray(previous_word)
    idx_all = np.ascontiguousarray(previous_word.reshape(B, 1).astype(np.int32))
    h_0 = np.asarray(h_0, dtype=f32)
    c_0 = np.asarray(c_0, dtype=f32)
    ctx = np.asarray(ctx, dtype=f32)
    mask_neg = np.where(np.asarray(ctx_mask), f32(NEG), f32(0.0)).astype(f32)
    emb = np.ascontiguousarray(np.asarray(emb, dtype=f32))
    W_ih = np.asarray(W_ih, dtype=f32)
    W_hh = np.asarray(W_hh, dtype=f32)
    # blocked layouts: [n, k, 128, 512], each block one contiguous DMA
    wih_blk = W_ih.T.reshape(NKI, P, NSPLIT, 512).transpose(2, 0, 1, 3)
    whh_blk = W_hh.T.reshape(NKH, P, NSPLIT, 512).transpose(2, 0, 1, 3)
    wg_blk = np.ascontiguousarray(np.concatenate([wih_blk, whh_blk], axis=1))
    bih = np.asarray(b_ih, dtype=f32).reshape(1, G4)
    bhh = np.asarray(b_hh, dtype=f32).reshape(1, G4)
    win_blk = np.ascontiguousarray(
        np.asarray(W_in, dtype=f32).T.reshape(NKH, P, 2, 512).transpose(2, 0, 1, 3))
    wout_blk = np.ascontiguousarray(
        np.asarray(W_out, dtype=f32).T.reshape(2 * NKH, P, 2, 512).transpose(2, 0, 1, 3))
    W_dec = np.asarray(W_dec, dtype=f32)
    b_dec = np.asarray(b_dec, dtype=f32)

    nc = _get_program()

    in_maps = []
    for c in range(NC):
        rows = slice(c * BC, (c + 1) * BC)
        vs = slice(c * VC, (c + 1) * VC)
        wdec_blk = np.ascontiguousarray(
            W_dec[vs].T.reshape(NKH, P, NV, NVW).transpose(2, 0, 1, 3))
        in_maps.append({
            "idx": idx_all[rows],
            "emb": emb,
            "h0T": np.ascontiguousarray(h_0[rows].T),
            "c0": np.ascontiguousarray(c_0[rows]),
            "ctxd": np.ascontiguousarray(ctx[rows]),
            "mnegT": np.ascontiguousarray(mask_neg[rows].T),
            "wg_blk": wg_blk,
            "bih": bih,
            "bhh": bhh,
            "win_blk": win_blk,
            "wout_blk": wout_blk,
            "wdec_blk": wdec_blk,
            "bdec": np.ascontiguousarray(b_dec[vs].reshape(1, VC)),
        })

    res = run_bass_kernel_spmd(nc, in_maps, list(range(NC)))
    last_results = res
    r = res.results
    h_1 = np.concatenate([r[c]["h1o"] for c in range(NC)], axis=0)
    c_1 = np.concatenate([r[c]["c1o"] for c in range(NC)], axis=0)
    alpha = np.concatenate([r[c]["alphao"] for c in range(NC)], axis=0)
    logit = np.concatenate([r[c]["logito"] for c in range(NC)], axis=1)
    return h_1, c_1, alpha, logit
